# revision 43
# baseline (speedup 1.0000x reference)
"""Trainium2 Bass kernel for nn_CpSae_44014824849572.

Computes the CP-SAE loss. The reference materializes a [1024, 64, 32, 32]
CP-reconstruction `volume` and diffs it against `features`. We instead use

  sum((flat - volume)^2) = sum(flat^2) - 2*sum(flat*volume) + sum(volume^2)

with  sum(flat*volume)[b] = sum_z a[b,z] * T[b,z],
      T[b,z]   = sum_feat flat[b,feat] * KRP[g_b][z,feat]
      KRP[g]   = softplus(freq)⊗softplus(roi1)⊗softplus(roi2)  (rank-1 rows)
      sum(volume^2)[b] = a_b^T M_{g_b} a_b,
      M_g = (Ff Ff^T) ∘ (R1 R1^T) ∘ (R2 R2^T)   (32x32 per group, tiny)

so the only heavy device work is two big contractions over the feature dim:
  zcat[b, 0:64] = flat[b] @ [W1 | W2]          (encoder, 8.6 GFLOP)
  T[b, z]       = flat[b] @ KRP[g_b].T         (4.3 GFLOP)

Distribution: feature-dim sharded across 8 cores (8192 features each = 8
frequency indices x 1024 roi-pairs). All matmuls run in fp8 DoubleRow mode
(256-deep contraction). The KRP stationary is factored:
KRP[g][z, (f,r,s)] = Ff[g,z,f] * R12[g,z,(r,s)]; the device contracts
against R12 only, with a QUAD-group stationary [256 x 128] covering 4
groups' 32 z-rows at once (samples are host-sorted by group so each
quad's T columns are contiguous). Per-f partial sums land in [128, 512]
PSUM banks and the Ff factor folds in on DVE during the PSUM drain:
T_acc += Ff'[f,:] * P_f (per-partition scalar multiply-add), where Ff' is
host-renormalized by the core's last frequency so the final fold is a
plain whole-tile psum+acc add (the host multiplies the Ff[f_last]/r_scale
factor back in when decoding tt).

The DMA stream is the bottleneck (9.4 MB/core at the model's 360 GB/s
aggregate ~ 26.3 us busy), so the kernel is organized around keeping the
single DMA_ENGINES resource saturated end-to-end and minimizing the
post-stream tail: few large flat transfers (each DMA costs ~625 ns on the
shared HWDGE issue path + 900 ns sem propagation), small tensors
interleaved on a second queue, the final frequency's 4 kps column-piped in
6 pieces so matmuls/folds/copies/output-DMAs overlap the last transfers,
and zcat leaving via the Pool/SWDGE path to dodge the HWDGE queue.
PSUM note: start_tensor_calc zeroes the whole psum bank (not just the
written region), so exactly one T-matmul per (bank, f) carries start=True.
Outputs return as per-core partial sums (tt bf16, zcat fp8 x W_SCALE)
that the host reduces across cores.
"""
import json

import numpy as np
import ml_dtypes

import concourse.bass as bass
import concourse.mybir as mybir
import concourse.tile as tile
from concourse.bass_utils import run_bass_kernel_spmd

N_CORES = 8
BATCH = 1024
N_FREQS = 64
N_ROIS = 32
Z = 32
N_GROUPS = 16
N_CLASSES = 4
N_FEAT = N_FREQS * N_ROIS * N_ROIS          # 65536
FEAT_PER_CORE = N_FEAT // N_CORES           # 8192
KPAIRS = FEAT_PER_CORE // 256               # 32 double-chunks of 256 features
F_PER_CORE = N_FREQS // N_CORES             # 8 frequency indices per core
RS = N_ROIS * N_ROIS                        # 1024 roi-pair features per f
N_TPS = 3                                   # T psum rotation depth
N_QUADS = 4                                 # group quads (4 groups x 32 z)
CB = 512                                    # left/right column split
EPSILON = 1e-06
REG_STRENGTH = 1.0
KL_FACTOR = 1.0

F32 = mybir.dt.float32
BF16 = mybir.dt.bfloat16
FP8 = mybir.dt.float8e4
NPFP8 = ml_dtypes.float8_e4m3
# zcat partials return as fp8: per-core partials are ~N(0, 0.2) with tails
# to ~+-1.02, so x128 stays inside the IEEE e4m3 +-240 range (1.8x margin)
# with ~6% quantization error that is negligible against the 2e-2 loss
# tolerance.
W_SCALE = 128.0
DR = mybir.MatmulPerfMode.DoubleRow
MULT = mybir.AluOpType.mult
ADD = mybir.AluOpType.add

# flat DMA chunks: (kp0, n_kp, col_off, col_width). Large transfers keep the
# shared HWDGE/SEQ issue overhead (~1.2us per DMA) off the critical path;
# chunks shrink toward the end (PE pays a 900ns DMA-sem latency per chunk,
# so small late chunks keep the post-stream backlog short).
FLAT_CHUNKS = [(0, 1, 0, BATCH), (1, 3, 0, BATCH)] + \
    [(4 * c, 4, 0, BATCH) for c in range(1, 6)] + [
    (24, 2, 0, BATCH),
    (26, 1, 0, BATCH),
    (27, 1, 0, BATCH),
]
# The last frequency's kps (28-31) stream as column pieces from a separate
# piece-major DRAM tensor (flat_tail) so each piece is one contiguous
# >=512B descriptor run per partition (sub-512B runs pay a 2x DMA latency
# penalty). Piecing the tail lets each piece's matmuls + fold-add + zc copy
# + output DMA overlap the next piece's transfer.
TAIL_KP0 = 28
TAIL_NKP = 4
TAIL_PIECES = [(0, 256), (256, CB), (CB, 704), (704, 832), (832, 928),
               (928, BATCH)]

_waitfix_counter = [0]


def _split_waits_in_bir(bir: dict) -> int:
    """This container's walrus accepts only ONE sync wait per instruction;
    Tile emits several. Hoist all-but-one wait onto EventSemaphore
    instructions inserted just before, on the same engine."""
    nsplit = 0
    for fn in bir.get("functions", []):
        for blk in fn.get("blocks", []):
            out = []
            for insn in blk.get("instructions", []):
                si = insn.get("sync_info") or {}
                ow = si.get("on_wait") or []
                if len(ow) > 1:
                    for w in ow[:-1]:
                        _waitfix_counter[0] += 1
                        out.append({
                            "debug": insn.get("debug", 0),
                            "engine": insn["engine"],
                            "ins": [],
                            "name": f"{insn['name']}-wsplit{_waitfix_counter[0]}",
                            "opcode": "EventSemaphore",
                            "outs": [],
                            "sync_info": {"on_update": [], "on_wait": [w]},
                        })
                        nsplit += 1
                    si["on_wait"] = [ow[-1]]
                out.append(insn)
            blk["instructions"] = out
    return nsplit


def _install_waitfix():
    import concourse.bass2jax as bass2jax
    import concourse.bass_utils as bass_utils

    if getattr(bass2jax, "_waitfix_installed", False):
        return
    orig = bass_utils.compile_bir_kernel

    def patched(bir_json, tmpdir, neff_name="file.neff"):
        bir = json.loads(bir_json.decode() if isinstance(bir_json, bytes) else bir_json)
        _split_waits_in_bir(bir)
        return orig(json.dumps(bir).encode(), tmpdir, neff_name)

    bass2jax.compile_bir_kernel = patched
    bass_utils.compile_bir_kernel = patched
    bass2jax._waitfix_installed = True


def _softplus(x):
    return np.logaddexp(0.0, x.astype(np.float64)).astype(np.float32)


def _quad_blocks(groups_sorted):
    """[(Q, c0, c1)] contiguous column blocks per group-quad Q (groups
    4Q..4Q+3). Blocks never straddle column CB (left/right psum tiles)
    and are therefore at most 512 wide (one psum bank)."""
    gs = np.asarray(groups_sorted)
    blocks = []
    for q in range(N_GROUPS // 4):
        c0 = int(np.searchsorted(gs, 4 * q))
        c1 = int(np.searchsorted(gs, 4 * q + 4))
        while c0 < c1:
            ce = min(c0 + 512, c1)
            if c0 < CB < ce:
                ce = CB
            blocks.append((q, c0, ce))
            c0 = ce
    return blocks


def build_device_program(blocks):
    """One SPMD program (shared by all 8 cores). Per-core inputs:
      flatt [128, KPAIRS, 2, BATCH] fp8 — feature slice, group-sorted columns;
                                          feature = kp*256 + sub*128 + p
      w     [128, KPAIRS, 2, 64]   fp8 — [W1|W2]*W_SCALE slice
      r12   [128, 4, 2, 512]       fp8 — R12*R_SCALE, rs = blk*256+sub*128+p,
                                          free axis is gz (group-major)
      ff    [128, 8, 4]            f32 — RENORMALIZED freq factors
                                          Ff[g,z,f]/Ff[g,z,f_last]: [i, f, Q]
                                          with g = 4Q+i//32, z = i%32. The
                                          last f's scale is identically 1, so
                                          its fold is a plain whole-tile add
                                          (host multiplies tt rows by
                                          Ff[g,z,f_last]/r_scale afterwards).
    Outputs (partial sums over this core's features):
      zcat [64, BATCH]  fp8  — encoder output [W1|W2] rows, x W_SCALE
      tt   [128, BATCH] bf16 — row 32*(g%4)+z holds T[z]*Ffnorm for that
                               column's group
    """
    nc = bass.Bass()
    flatt = nc.dram_tensor("flatt", [128, KPAIRS, 2, BATCH], FP8, kind="ExternalInput")
    flat_tail = nc.dram_tensor("flat_tail", [128, TAIL_NKP * 2 * BATCH], FP8,
                               kind="ExternalInput")
    w = nc.dram_tensor("w", [128, KPAIRS, 2, 64], FP8, kind="ExternalInput")
    r12 = nc.dram_tensor("r12", [128, 4, 2, 512], FP8, kind="ExternalInput")
    ff = nc.dram_tensor("ff", [128, F_PER_CORE, 4], F32, kind="ExternalInput")
    zcat_out = nc.dram_tensor("zcat", [64, BATCH], FP8, kind="ExternalOutput")
    tt_out = nc.dram_tensor("tt", [128, BATCH], BF16, kind="ExternalOutput")

    with tile.TileContext(nc) as tc:
        with (
            tc.tile_pool(name="fpool", bufs=1) as fpool,
            tc.tile_pool(name="const", bufs=1) as const,
            tc.tile_pool(name="psum", bufs=1, space="PSUM") as psum,
        ):
            # --- input DMAs ---
            # scalar(Act) SEQ carries the small tensors; sync(SP) SEQ carries
            # the flat stream. Transfers interleave on the shared DMA engines
            # in issue order, so w and r12 land within the first few chunks.
            wt = const.tile([128, KPAIRS, 2, 64], FP8, tag="w")
            r12t = const.tile([128, 4, 2, 512], FP8, tag="r12")
            fft = const.tile([128, F_PER_CORE, 4], F32, tag="ff")
            nc.scalar.dma_start(out=wt, in_=w[:, :, :, :])
            nc.scalar.dma_start(out=r12t, in_=r12[:, :, :, :])
            nc.scalar.dma_start(out=fft, in_=ff[:, :, :])

            flat_tiles = []  # (kp0, n_kp, col_off, width, tile)
            for ci, (kp0, nk, off, wd) in enumerate(FLAT_CHUNKS):
                t = fpool.tile([128, nk, 2, wd], FP8, tag=f"flat{ci}",
                               name=f"flat{ci}")
                flat_tiles.append((kp0, nk, off, wd, t))
                nc.sync.dma_start(out=t, in_=flatt[:, kp0:kp0 + nk, :, off:off + wd])
            toff = 0
            for pi, (p0, p1) in enumerate(TAIL_PIECES):
                pw = p1 - p0
                t = fpool.tile([128, TAIL_NKP, 2, pw], FP8, tag=f"ftail{pi}",
                               name=f"ftail{pi}")
                flat_tiles.append((TAIL_KP0, TAIL_NKP, p0, pw, t))
                nc.sync.dma_start(
                    out=t, in_=flat_tail[:, toff:toff + 2 * TAIL_NKP * pw])
                toff += 2 * TAIL_NKP * pw

            def ft_rhs(kp, c0, c1):
                for kp0, nk, off, wd, t in flat_tiles:
                    if kp0 <= kp < kp0 + nk and off <= c0 and c1 <= off + wd:
                        return t[:, kp - kp0, :, c0 - off:c1 - off]
                raise AssertionError((kp, c0, c1))

            # Side-wide T accumulators (columns 0:512 / 512:1024); f<7 folds
            # write per-block slices, the last fold is one whole-tile add.
            # Separate left/right tiles keep the framework's whole-tile
            # dependency tracking from serializing the two drain chains.
            tacc_l = const.tile([128, CB], F32, tag="tacc_l")
            tacc_r = const.tile([128, BATCH - CB], F32, tag="tacc_r")
            tout_l = const.tile([128, CB], BF16, tag="tout_l")
            tout_r = const.tile([128, BATCH - CB], BF16, tag="tout_r")
            zc_sb_l = const.tile([64, CB], FP8, tag="zc_sb_l")
            zc_sb_r = const.tile([64, BATCH - CB], FP8, tag="zc_sb_r")

            zc_ps = [psum.tile([64, 512], F32, tag=f"zcp{h}", name=f"zcp{h}")
                     for h in range(2)]
            # T psum split left/right (columns 0:512 vs 512:1024), one full
            # [128, 512] bank each, rotated over N_TPS f-values.
            t_ps = []
            for i in range(N_TPS):
                ta = psum.tile([128, CB], F32, tag=f"ta{i}", name=f"ta{i}")
                tb = psum.tile([128, BATCH - CB], F32, tag=f"tb{i}", name=f"tb{i}")
                t_ps.append((ta, tb))

            def enc_mm(kp, c0, c1, start, stop, skip=False):
                h = 0 if c1 <= CB else 1
                nc.tensor.matmul(
                    zc_ps[h][:, c0 - h * CB:c1 - h * CB],
                    wt[:, kp, :, :],
                    ft_rhs(kp, c0, c1),
                    start=start, stop=stop,
                    perf_mode=DR, skip_group_check=skip)

            # start_tensor_calc zeroes the ENTIRE psum bank, not just the
            # written region (verified on hardware), so exactly ONE T-matmul
            # per (psum tile, f) may carry start=True: the side-first block's
            # first-kp matmul. It zeroes the whole bank; every other block's
            # matmuls accumulate onto those zeros.
            first_left = next(b for b in blocks if b[2] <= CB)
            first_right = next(b for b in blocks if b[2] > CB)

            def t_mm(kp, c_lo, c_hi):
                f, blk = divmod(kp, 4)
                tpa, tpb = t_ps[f % N_TPS]
                for (q, b0, b1) in blocks:
                    c0, c1 = max(b0, c_lo), min(b1, c_hi)
                    if c0 >= c1:
                        continue
                    left = c1 <= CB
                    tp, ob = (tpa, 0) if left else (tpb, CB)
                    first = (q, b0, b1) == (first_left if left else first_right)
                    nc.tensor.matmul(
                        tp[:, c0 - ob:c1 - ob],
                        r12t[:, blk, :, 128 * q:128 * (q + 1)],
                        ft_rhs(kp, c0, c1),
                        start=(blk == 0 and first and c0 == b0),
                        stop=(blk == 3),
                        perf_mode=DR, skip_group_check=True)

            def fold(f, blist):
                # T_acc += Ffnorm[f] * P_f on DVE, straight from psum,
                # per-partition scalar = renormalized Ff for this (f, quad).
                # The last f's scale is 1.0 (host renormalization), so its
                # "fold" is a single whole-tile add per side — the only fold
                # on the post-stream critical path.
                tpa, tpb = t_ps[f % N_TPS]
                for (q, c0, c1) in blist:
                    left = c1 <= CB
                    tp, ob = (tpa, 0) if left else (tpb, CB)
                    src = tp[:, c0 - ob:c1 - ob]
                    sc = fft[:, f, q:q + 1]
                    tacc = tacc_l if left else tacc_r
                    ta = tacc[:, c0 - ob:c1 - ob]
                    if f == 0:
                        nc.vector.tensor_scalar(ta, src, sc, None, MULT)
                    else:
                        nc.vector.scalar_tensor_tensor(ta, src, sc, ta, MULT, ADD)

            # kps 0..27 arrive full-width
            for kp in range(TAIL_KP0):
                enc_mm(kp, 0, CB, start=(kp == 0), stop=False)
                enc_mm(kp, CB, BATCH, start=(kp == 0), stop=False)
                t_mm(kp, 0, BATCH)
                f, blk = divmod(kp, 4)
                if blk == 3:
                    fold(f, blocks)
            # The last frequency's kps (28-31) arrive as column pieces
            # (0:512, 512:896, 896:1024 — all 4 kps per piece): each piece's
            # matmuls, final fold-add, zc copy, and output DMA run while
            # later pieces are still in flight, so only the tiny last
            # piece's chain sits after the stream. The piece matmuls write
            # sub-ranges of the full-block psum regions, so the tile group
            # check is skipped (element-wise accumulation is still
            # well-formed: first write has start, last has stop).
            # Per-piece matmuls; the fold-add + zc copy + output DMA flush
            # once per SIDE (after the last piece of that side) — a per-piece
            # add would serialize the next piece's matmuls through the
            # whole-tile psum dependency.
            tpa, tpb = t_ps[((TAIL_KP0 // 4)) % N_TPS]
            last_kp = TAIL_KP0 + TAIL_NKP - 1
            for (p0, p1) in TAIL_PIECES:
                for kp in range(TAIL_KP0, TAIL_KP0 + TAIL_NKP):
                    t_mm(kp, p0, p1)
                    enc_mm(kp, p0, p1, start=False, stop=(kp == last_kp),
                           skip=True)
                # zcat leaves via the Pool/SWDGE path: it bypasses the
                # shared HWDGE issue queue, which the two tt DMAs occupy at
                # the tail; per-side DMAs let the left half leave early
                # (separate zc_sb tiles keep the whole-tile dep tracker from
                # chaining the left DMA behind the right copy).
                if p1 == CB:
                    nc.vector.tensor_tensor(
                        tout_l[:, :], tpa[:, :], tacc_l[:, :], ADD)
                    nc.scalar.copy(zc_sb_l[:, :], zc_ps[0][:, :])
                    nc.sync.dma_start(out=tt_out[:, 0:CB], in_=tout_l[:, :])
                    nc.gpsimd.dma_start(out=zcat_out[:, 0:CB],
                                        in_=zc_sb_l[:, :])
                elif p1 == BATCH:
                    nc.vector.tensor_tensor(
                        tout_r[:, :], tpb[:, :], tacc_r[:, :], ADD)
                    nc.scalar.copy(zc_sb_r[:, :], zc_ps[1][:, :])
                    nc.sync.dma_start(out=tt_out[:, CB:BATCH], in_=tout_r[:, :])
                    nc.gpsimd.dma_start(out=zcat_out[:, CB:BATCH],
                                        in_=zc_sb_r[:, :])
    return nc


def _prepare(inputs):
    features = np.asarray(inputs["features"], dtype=np.float32)
    labels = np.asarray(inputs["labels"]).astype(np.int64)
    groups = np.asarray(inputs["groups"]).astype(np.int64)
    weights = np.asarray(inputs["weights"], dtype=np.float32)
    noise = np.asarray(inputs["noise"], dtype=np.float32)
    group_embed = np.asarray(inputs["group_embed"], dtype=np.float32)
    W1 = np.asarray(inputs["W1"], dtype=np.float32)
    b1 = np.asarray(inputs["b1"], dtype=np.float32)
    W2 = np.asarray(inputs["W2"], dtype=np.float32)
    b2 = np.asarray(inputs["b2"], dtype=np.float32)
    freq_factors = np.asarray(inputs["freq_factors"], dtype=np.float32)
    roi_1_factors = np.asarray(inputs["roi_1_factors"], dtype=np.float32)
    roi_2_factors = np.asarray(inputs["roi_2_factors"], dtype=np.float32)
    lin_W = np.asarray(inputs["lin_W"], dtype=np.float32)
    lin_b = np.asarray(inputs["lin_b"], dtype=np.float32)
    logit_bias = np.asarray(inputs["logit_bias"], dtype=np.float32)

    b = features.shape[0]
    flat = features.reshape(b, -1)

    perm = np.argsort(groups, kind="stable")
    groups_sorted = groups[perm]
    blocks = _quad_blocks(groups_sorted)

    sq = np.einsum("bi,bi->b", flat, flat, optimize=True)

    flat_q = flat[perm].astype(NPFP8)
    flatT = flat_q.view(np.uint8).T.copy().view(NPFP8)      # [N_FEAT, BATCH]

    W = (np.concatenate([W1[:N_FEAT], W2[:N_FEAT]], axis=1) * W_SCALE).astype(NPFP8)

    Ff = _softplus(freq_factors)
    R1 = _softplus(roi_1_factors)
    R2 = _softplus(roi_2_factors)

    # R12[(r,s), (g,z)] in fp8 with a dynamic power-of-two scale.
    R12 = (R1[:, :, :, None] * R2[:, :, None, :])           # [g, z, r, s]
    # ml_dtypes.float8_e4m3 is the IEEE variant: max finite value is 240
    r_scale = float(2.0 ** np.floor(np.log2(224.0 / max(float(R12.max()), 1e-6))))
    R12q = (R12 * r_scale).transpose(2, 3, 0, 1).reshape(RS, N_GROUPS * Z)
    R12q = R12q.astype(NPFP8)                               # [rs, gz]
    r12_dev = R12q.view(np.uint8).reshape(4, 2, 128, N_GROUPS * Z)
    r12_dev = r12_dev.transpose(2, 0, 1, 3).copy().view(NPFP8)

    # ff[i, f, Q] = Ff[g, z, f]/Ff[g, z, f_last] with g = 4Q+i//32, z = i%32
    # (renormalized so the device's last fold is a plain add; the host folds
    # the Ff[g, z, f_last]/r_scale factor back in when decoding tt). f_last
    # is per-core: the core's highest frequency index.
    ff_all = np.zeros((N_CORES, 128, F_PER_CORE, N_QUADS), np.float32)
    ff_last = np.zeros((N_CORES, N_GROUPS, Z), np.float32)
    for c in range(N_CORES):
        fglob = np.arange(F_PER_CORE) + c * F_PER_CORE
        ff_last[c] = Ff[:, :, fglob[-1]]
        for q in range(N_GROUPS // 4):
            for gp in range(4):
                g = 4 * q + gp
                p0 = gp * 32
                # Ff[g, :, fglob] has shape (f, z) via advanced indexing
                ff_all[c, p0:p0 + 32, :, q] = (
                    Ff[g, :, fglob] / ff_last[c, g][None, :]).T

    in_maps = []
    for c in range(N_CORES):
        fsl = flatT[c * FEAT_PER_CORE:(c + 1) * FEAT_PER_CORE]
        fsl = fsl.view(np.uint8).reshape(KPAIRS, 2, 128, BATCH)
        fsl = fsl.transpose(2, 0, 1, 3).copy().view(NPFP8)
        # piece-major layout of the tail kps for the contiguous tail DMAs
        tail = fsl[:, TAIL_KP0:TAIL_KP0 + TAIL_NKP, :, :].view(np.uint8)
        tail = np.concatenate(
            [tail[:, :, :, p0:p1].reshape(128, -1) for (p0, p1) in TAIL_PIECES],
            axis=1).copy().view(NPFP8)
        wsl = W[c * FEAT_PER_CORE:(c + 1) * FEAT_PER_CORE].view(np.uint8)
        wsl = wsl.reshape(KPAIRS, 2, 128, 64).transpose(2, 0, 1, 3).copy().view(NPFP8)
        in_maps.append({
            "flatt": fsl,
            "flat_tail": tail,
            "w": wsl,
            "r12": r12_dev,
            "ff": ff_all[c],
        })

    host = dict(
        labels=labels, groups=groups, weights=weights, noise=noise,
        group_embed=group_embed, W1=W1, b1=b1, W2=W2, b2=b2,
        lin_W=lin_W, lin_b=lin_b, logit_bias=logit_bias,
        Ff=Ff, R1=R1, R2=R2, sq=sq, perm=perm, b=b,
        groups_sorted=groups_sorted, ff_last=ff_last, r_scale=r_scale,
    )
    return in_maps, blocks, host


def _finish(zcatT, T_sorted, host):
    b = host["b"]
    perm = host["perm"]
    inv = np.empty_like(perm)
    inv[perm] = np.arange(b)

    zcat = (zcatT / W_SCALE).T[inv]                        # [b, 64]
    T = T_sorted[inv]

    groups = host["groups"]
    ge = host["group_embed"][groups]
    z_mu = zcat[:, :Z] + host["b1"] + ge @ host["W1"][N_FEAT:]
    z_log_std = zcat[:, Z:] + host["b2"] + ge @ host["W2"][N_FEAT:]
    sigma = EPSILON + np.exp(z_log_std)
    kld = np.sum(-np.log(sigma) + 0.5 * (sigma * sigma + z_mu * z_mu - 1.0), axis=1)
    zs = z_mu + sigma * host["noise"]
    zs = zs @ host["lin_W"] + host["lin_b"]
    a = _softplus(zs)

    Ff, R1, R2 = host["Ff"], host["R1"], host["R2"]
    M = (np.einsum("gzf,gyf->gzy", Ff, Ff)
         * np.einsum("gzr,gyr->gzy", R1, R1)
         * np.einsum("gzs,gys->gzy", R2, R2))
    vol2 = np.einsum("bz,bzy,by->b", a, M[groups], a)
    fdotv = np.sum(a * T, axis=1)
    rec = REG_STRENGTH * (host["sq"] - 2.0 * fdotv + vol2) / N_FEAT

    logits = np.concatenate([zs[:, :N_CLASSES - 1], np.ones((b, 1), np.float32)],
                            axis=1) + host["logit_bias"]
    m = logits.max(axis=1, keepdims=True)
    lse = m[:, 0] + np.log(np.exp(logits - m).sum(axis=1))
    log_probs = logits[np.arange(b), host["labels"]] - lse

    freq_loss = np.var(Ff, axis=0, ddof=1).mean(axis=1).sum()
    roi_loss = (np.var(R1, axis=0, ddof=1) + np.var(R2, axis=0, ddof=1)).mean(axis=1).sum()

    loss = np.mean(rec - host["weights"] * log_probs + KL_FACTOR * kld) \
        + freq_loss + roi_loss
    return np.float32(loss)


def kernel(**inputs) -> np.ndarray:
    _install_waitfix()
    in_maps, blocks, host = _prepare(inputs)
    nc = build_device_program(blocks)
    r = run_bass_kernel_spmd(nc, in_maps, core_ids=list(range(N_CORES)))
    gs = host["groups_sorted"]
    gs4 = (gs % 4).astype(np.int64)
    cols = np.arange(host["b"])
    zcatT = np.zeros((64, BATCH), np.float32)
    T_sorted = np.zeros((host["b"], Z), np.float32)
    for c in range(N_CORES):
        zcatT += r.results[c]["zcat"].astype(np.float32)
        ttc = r.results[c]["tt"].astype(np.float32)
        # undo the per-core Ff renormalization: x Ff[g, z, f_last]/r_scale
        scale = host["ff_last"][c][gs]                     # [b, Z]
        for zi in range(Z):
            T_sorted[:, zi] += ttc[gs4 * Z + zi, cols] * scale[:, zi]
    T_sorted /= host["r_scale"]
    return _finish(zcatT, T_sorted, host)


# revision 44
# speedup vs baseline: 1.0008x; 1.0008x over previous
"""Trainium2 Bass kernel for nn_CpSae_44014824849572.

Computes the CP-SAE loss. The reference materializes a [1024, 64, 32, 32]
CP-reconstruction `volume` and diffs it against `features`. We instead use

  sum((flat - volume)^2) = sum(flat^2) - 2*sum(flat*volume) + sum(volume^2)

with  sum(flat*volume)[b] = sum_z a[b,z] * T[b,z],
      T[b,z]   = sum_feat flat[b,feat] * KRP[g_b][z,feat]
      KRP[g]   = softplus(freq)⊗softplus(roi1)⊗softplus(roi2)  (rank-1 rows)
      sum(volume^2)[b] = a_b^T M_{g_b} a_b,
      M_g = (Ff Ff^T) ∘ (R1 R1^T) ∘ (R2 R2^T)   (32x32 per group, tiny)

so the only heavy device work is two big contractions over the feature dim:
  zcat[b, 0:64] = flat[b] @ [W1 | W2]          (encoder, 8.6 GFLOP)
  T[b, z]       = flat[b] @ KRP[g_b].T         (4.3 GFLOP)

Distribution: feature-dim sharded across 8 cores (8192 features each = 8
frequency indices x 1024 roi-pairs). All matmuls run in fp8 DoubleRow mode
(256-deep contraction). The KRP stationary is factored:
KRP[g][z, (f,r,s)] = Ff[g,z,f] * R12[g,z,(r,s)]; the device contracts
against R12 only, with a QUAD-group stationary [256 x 128] covering 4
groups' 32 z-rows at once (samples are host-sorted by group so each
quad's T columns are contiguous). Per-f partial sums land in [128, 512]
PSUM banks and the Ff factor folds in on DVE during the PSUM drain:
T_acc += Ff'[f,:] * P_f (per-partition scalar multiply-add), where Ff' is
host-renormalized by the core's last frequency so the final fold is a
plain whole-tile psum+acc add (the host multiplies the Ff[f_last]/r_scale
factor back in when decoding tt).

The DMA stream is the bottleneck (9.4 MB/core at the model's 360 GB/s
aggregate ~ 26.3 us busy), so the kernel is organized around keeping the
single DMA_ENGINES resource saturated end-to-end and minimizing the
post-stream tail: few large flat transfers (each DMA costs ~625 ns on the
shared HWDGE issue path + 900 ns sem propagation), small tensors
interleaved on a second queue, the final frequency's 4 kps column-piped in
6 pieces so matmuls/folds/copies/output-DMAs overlap the last transfers,
and zcat leaving via the Pool/SWDGE path to dodge the HWDGE queue.
PSUM note: start_tensor_calc zeroes the whole psum bank (not just the
written region), so exactly one T-matmul per (bank, f) carries start=True.
Outputs return as per-core partial sums (tt bf16, zcat fp8 x W_SCALE)
that the host reduces across cores.
"""
import json

import numpy as np
import ml_dtypes

import concourse.bass as bass
import concourse.mybir as mybir
import concourse.tile as tile
from concourse.bass_utils import run_bass_kernel_spmd

N_CORES = 8
BATCH = 1024
N_FREQS = 64
N_ROIS = 32
Z = 32
N_GROUPS = 16
N_CLASSES = 4
N_FEAT = N_FREQS * N_ROIS * N_ROIS          # 65536
FEAT_PER_CORE = N_FEAT // N_CORES           # 8192
KPAIRS = FEAT_PER_CORE // 256               # 32 double-chunks of 256 features
F_PER_CORE = N_FREQS // N_CORES             # 8 frequency indices per core
RS = N_ROIS * N_ROIS                        # 1024 roi-pair features per f
N_TPS = 3                                   # T psum rotation depth
N_QUADS = 4                                 # group quads (4 groups x 32 z)
CB = 512                                    # left/right column split
EPSILON = 1e-06
REG_STRENGTH = 1.0
KL_FACTOR = 1.0

F32 = mybir.dt.float32
BF16 = mybir.dt.bfloat16
FP8 = mybir.dt.float8e4
NPFP8 = ml_dtypes.float8_e4m3
# zcat partials return as fp8: per-core partials are ~N(0, 0.2) with tails
# to ~+-1.02, so x128 stays inside the IEEE e4m3 +-240 range (1.8x margin)
# with ~6% quantization error that is negligible against the 2e-2 loss
# tolerance.
W_SCALE = 128.0
DR = mybir.MatmulPerfMode.DoubleRow
MULT = mybir.AluOpType.mult
ADD = mybir.AluOpType.add

# flat DMA chunks: (kp0, n_kp, col_off, col_width). Large transfers keep the
# shared HWDGE/SEQ issue overhead (~1.2us per DMA) off the critical path;
# chunks shrink toward the end (PE pays a 900ns DMA-sem latency per chunk,
# so small late chunks keep the post-stream backlog short).
FLAT_CHUNKS = [(0, 1, 0, BATCH), (1, 3, 0, BATCH)] + \
    [(4 * c, 4, 0, BATCH) for c in range(1, 6)] + [
    (24, 2, 0, BATCH),
    (26, 1, 0, BATCH),
    (27, 1, 0, BATCH),
]
# The last frequency's kps (28-31) stream as column pieces from a separate
# piece-major DRAM tensor (flat_tail) so each piece is one contiguous
# >=512B descriptor run per partition (sub-512B runs pay a 2x DMA latency
# penalty). Piecing the tail lets each piece's matmuls + fold-add + zc copy
# + output DMA overlap the next piece's transfer.
TAIL_KP0 = 28
TAIL_NKP = 4
TAIL_PIECES = [(0, 256), (256, CB), (CB, 704), (704, 832), (832, 960),
               (960, BATCH)]

_waitfix_counter = [0]


def _split_waits_in_bir(bir: dict) -> int:
    """This container's walrus accepts only ONE sync wait per instruction;
    Tile emits several. Hoist all-but-one wait onto EventSemaphore
    instructions inserted just before, on the same engine."""
    nsplit = 0
    for fn in bir.get("functions", []):
        for blk in fn.get("blocks", []):
            out = []
            for insn in blk.get("instructions", []):
                si = insn.get("sync_info") or {}
                ow = si.get("on_wait") or []
                if len(ow) > 1:
                    for w in ow[:-1]:
                        _waitfix_counter[0] += 1
                        out.append({
                            "debug": insn.get("debug", 0),
                            "engine": insn["engine"],
                            "ins": [],
                            "name": f"{insn['name']}-wsplit{_waitfix_counter[0]}",
                            "opcode": "EventSemaphore",
                            "outs": [],
                            "sync_info": {"on_update": [], "on_wait": [w]},
                        })
                        nsplit += 1
                    si["on_wait"] = [ow[-1]]
                out.append(insn)
            blk["instructions"] = out
    return nsplit


def _install_waitfix():
    import concourse.bass2jax as bass2jax
    import concourse.bass_utils as bass_utils

    if getattr(bass2jax, "_waitfix_installed", False):
        return
    orig = bass_utils.compile_bir_kernel

    def patched(bir_json, tmpdir, neff_name="file.neff"):
        bir = json.loads(bir_json.decode() if isinstance(bir_json, bytes) else bir_json)
        _split_waits_in_bir(bir)
        return orig(json.dumps(bir).encode(), tmpdir, neff_name)

    bass2jax.compile_bir_kernel = patched
    bass_utils.compile_bir_kernel = patched
    bass2jax._waitfix_installed = True


def _softplus(x):
    return np.logaddexp(0.0, x.astype(np.float64)).astype(np.float32)


def _quad_blocks(groups_sorted):
    """[(Q, c0, c1)] contiguous column blocks per group-quad Q (groups
    4Q..4Q+3). Blocks never straddle column CB (left/right psum tiles)
    and are therefore at most 512 wide (one psum bank)."""
    gs = np.asarray(groups_sorted)
    blocks = []
    for q in range(N_GROUPS // 4):
        c0 = int(np.searchsorted(gs, 4 * q))
        c1 = int(np.searchsorted(gs, 4 * q + 4))
        while c0 < c1:
            ce = min(c0 + 512, c1)
            if c0 < CB < ce:
                ce = CB
            blocks.append((q, c0, ce))
            c0 = ce
    return blocks


def build_device_program(blocks):
    """One SPMD program (shared by all 8 cores). Per-core inputs:
      flatt [128, KPAIRS, 2, BATCH] fp8 — feature slice, group-sorted columns;
                                          feature = kp*256 + sub*128 + p
      w     [128, KPAIRS, 2, 64]   fp8 — [W1|W2]*W_SCALE slice
      r12   [128, 4, 2, 512]       fp8 — R12*R_SCALE, rs = blk*256+sub*128+p,
                                          free axis is gz (group-major)
      ff    [128, 8, 4]            f32 — RENORMALIZED freq factors
                                          Ff[g,z,f]/Ff[g,z,f_last]: [i, f, Q]
                                          with g = 4Q+i//32, z = i%32. The
                                          last f's scale is identically 1, so
                                          its fold is a plain whole-tile add
                                          (host multiplies tt rows by
                                          Ff[g,z,f_last]/r_scale afterwards).
    Outputs (partial sums over this core's features):
      zcat [64, BATCH]  fp8  — encoder output [W1|W2] rows, x W_SCALE
      tt   [128, BATCH] bf16 — row 32*(g%4)+z holds T[z]*Ffnorm for that
                               column's group
    """
    nc = bass.Bass()
    flatt = nc.dram_tensor("flatt", [128, KPAIRS, 2, BATCH], FP8, kind="ExternalInput")
    flat_tail = nc.dram_tensor("flat_tail", [128, TAIL_NKP * 2 * BATCH], FP8,
                               kind="ExternalInput")
    w = nc.dram_tensor("w", [128, KPAIRS, 2, 64], FP8, kind="ExternalInput")
    r12 = nc.dram_tensor("r12", [128, 4, 2, 512], FP8, kind="ExternalInput")
    ff = nc.dram_tensor("ff", [128, F_PER_CORE, 4], F32, kind="ExternalInput")
    zcat_out = nc.dram_tensor("zcat", [64, BATCH], FP8, kind="ExternalOutput")
    tt_out = nc.dram_tensor("tt", [128, BATCH], BF16, kind="ExternalOutput")

    with tile.TileContext(nc) as tc:
        with (
            tc.tile_pool(name="fpool", bufs=1) as fpool,
            tc.tile_pool(name="const", bufs=1) as const,
            tc.tile_pool(name="psum", bufs=1, space="PSUM") as psum,
        ):
            # --- input DMAs ---
            # scalar(Act) SEQ carries the small tensors; sync(SP) SEQ carries
            # the flat stream. Transfers interleave on the shared DMA engines
            # in issue order, so w and r12 land within the first few chunks.
            wt = const.tile([128, KPAIRS, 2, 64], FP8, tag="w")
            r12t = const.tile([128, 4, 2, 512], FP8, tag="r12")
            fft = const.tile([128, F_PER_CORE, 4], F32, tag="ff")
            nc.scalar.dma_start(out=wt, in_=w[:, :, :, :])
            nc.scalar.dma_start(out=r12t, in_=r12[:, :, :, :])
            nc.scalar.dma_start(out=fft, in_=ff[:, :, :])

            flat_tiles = []  # (kp0, n_kp, col_off, width, tile)
            for ci, (kp0, nk, off, wd) in enumerate(FLAT_CHUNKS):
                t = fpool.tile([128, nk, 2, wd], FP8, tag=f"flat{ci}",
                               name=f"flat{ci}")
                flat_tiles.append((kp0, nk, off, wd, t))
                nc.sync.dma_start(out=t, in_=flatt[:, kp0:kp0 + nk, :, off:off + wd])
            toff = 0
            for pi, (p0, p1) in enumerate(TAIL_PIECES):
                pw = p1 - p0
                t = fpool.tile([128, TAIL_NKP, 2, pw], FP8, tag=f"ftail{pi}",
                               name=f"ftail{pi}")
                flat_tiles.append((TAIL_KP0, TAIL_NKP, p0, pw, t))
                nc.sync.dma_start(
                    out=t, in_=flat_tail[:, toff:toff + 2 * TAIL_NKP * pw])
                toff += 2 * TAIL_NKP * pw

            def ft_rhs(kp, c0, c1):
                for kp0, nk, off, wd, t in flat_tiles:
                    if kp0 <= kp < kp0 + nk and off <= c0 and c1 <= off + wd:
                        return t[:, kp - kp0, :, c0 - off:c1 - off]
                raise AssertionError((kp, c0, c1))

            # Side-wide T accumulators (columns 0:512 / 512:1024); f<7 folds
            # write per-block slices, the last fold is one whole-tile add.
            # Separate left/right tiles keep the framework's whole-tile
            # dependency tracking from serializing the two drain chains.
            tacc_l = const.tile([128, CB], F32, tag="tacc_l")
            tacc_r = const.tile([128, BATCH - CB], F32, tag="tacc_r")
            tout_l = const.tile([128, CB], BF16, tag="tout_l")
            tout_r = const.tile([128, BATCH - CB], BF16, tag="tout_r")
            zc_sb_l = const.tile([64, CB], FP8, tag="zc_sb_l")
            zc_sb_r = const.tile([64, BATCH - CB], FP8, tag="zc_sb_r")

            zc_ps = [psum.tile([64, 512], F32, tag=f"zcp{h}", name=f"zcp{h}")
                     for h in range(2)]
            # T psum split left/right (columns 0:512 vs 512:1024), one full
            # [128, 512] bank each, rotated over N_TPS f-values.
            t_ps = []
            for i in range(N_TPS):
                ta = psum.tile([128, CB], F32, tag=f"ta{i}", name=f"ta{i}")
                tb = psum.tile([128, BATCH - CB], F32, tag=f"tb{i}", name=f"tb{i}")
                t_ps.append((ta, tb))

            def enc_mm(kp, c0, c1, start, stop, skip=False):
                h = 0 if c1 <= CB else 1
                nc.tensor.matmul(
                    zc_ps[h][:, c0 - h * CB:c1 - h * CB],
                    wt[:, kp, :, :],
                    ft_rhs(kp, c0, c1),
                    start=start, stop=stop,
                    perf_mode=DR, skip_group_check=skip)

            # start_tensor_calc zeroes the ENTIRE psum bank, not just the
            # written region (verified on hardware), so exactly ONE T-matmul
            # per (psum tile, f) may carry start=True: the side-first block's
            # first-kp matmul. It zeroes the whole bank; every other block's
            # matmuls accumulate onto those zeros.
            first_left = next(b for b in blocks if b[2] <= CB)
            first_right = next(b for b in blocks if b[2] > CB)

            def t_mm(kp, c_lo, c_hi):
                f, blk = divmod(kp, 4)
                tpa, tpb = t_ps[f % N_TPS]
                for (q, b0, b1) in blocks:
                    c0, c1 = max(b0, c_lo), min(b1, c_hi)
                    if c0 >= c1:
                        continue
                    left = c1 <= CB
                    tp, ob = (tpa, 0) if left else (tpb, CB)
                    first = (q, b0, b1) == (first_left if left else first_right)
                    nc.tensor.matmul(
                        tp[:, c0 - ob:c1 - ob],
                        r12t[:, blk, :, 128 * q:128 * (q + 1)],
                        ft_rhs(kp, c0, c1),
                        start=(blk == 0 and first and c0 == b0),
                        stop=(blk == 3),
                        perf_mode=DR, skip_group_check=True)

            def fold(f, blist):
                # T_acc += Ffnorm[f] * P_f on DVE, straight from psum,
                # per-partition scalar = renormalized Ff for this (f, quad).
                # The last f's scale is 1.0 (host renormalization), so its
                # "fold" is a single whole-tile add per side — the only fold
                # on the post-stream critical path.
                tpa, tpb = t_ps[f % N_TPS]
                for (q, c0, c1) in blist:
                    left = c1 <= CB
                    tp, ob = (tpa, 0) if left else (tpb, CB)
                    src = tp[:, c0 - ob:c1 - ob]
                    sc = fft[:, f, q:q + 1]
                    tacc = tacc_l if left else tacc_r
                    ta = tacc[:, c0 - ob:c1 - ob]
                    if f == 0:
                        nc.vector.tensor_scalar(ta, src, sc, None, MULT)
                    else:
                        nc.vector.scalar_tensor_tensor(ta, src, sc, ta, MULT, ADD)

            # kps 0..27 arrive full-width
            for kp in range(TAIL_KP0):
                enc_mm(kp, 0, CB, start=(kp == 0), stop=False)
                enc_mm(kp, CB, BATCH, start=(kp == 0), stop=False)
                t_mm(kp, 0, BATCH)
                f, blk = divmod(kp, 4)
                if blk == 3:
                    fold(f, blocks)
            # The last frequency's kps (28-31) arrive as column pieces
            # (0:512, 512:896, 896:1024 — all 4 kps per piece): each piece's
            # matmuls, final fold-add, zc copy, and output DMA run while
            # later pieces are still in flight, so only the tiny last
            # piece's chain sits after the stream. The piece matmuls write
            # sub-ranges of the full-block psum regions, so the tile group
            # check is skipped (element-wise accumulation is still
            # well-formed: first write has start, last has stop).
            # Per-piece matmuls; the fold-add + zc copy + output DMA flush
            # once per SIDE (after the last piece of that side) — a per-piece
            # add would serialize the next piece's matmuls through the
            # whole-tile psum dependency.
            tpa, tpb = t_ps[((TAIL_KP0 // 4)) % N_TPS]
            last_kp = TAIL_KP0 + TAIL_NKP - 1
            for (p0, p1) in TAIL_PIECES:
                for kp in range(TAIL_KP0, TAIL_KP0 + TAIL_NKP):
                    # enc first: the zc copy chain is gated by the last
                    # piece's enc, the fold-add by its T matmuls — this
                    # order balances the two output chains.
                    enc_mm(kp, p0, p1, start=False, stop=(kp == last_kp),
                           skip=True)
                    t_mm(kp, p0, p1)
                # zcat leaves via the Pool/SWDGE path: it bypasses the
                # shared HWDGE issue queue, which the two tt DMAs occupy at
                # the tail; per-side DMAs let the left half leave early
                # (separate zc_sb tiles keep the whole-tile dep tracker from
                # chaining the left DMA behind the right copy).
                if p1 == CB:
                    nc.vector.tensor_tensor(
                        tout_l[:, :], tpa[:, :], tacc_l[:, :], ADD)
                    nc.scalar.copy(zc_sb_l[:, :], zc_ps[0][:, :])
                    nc.sync.dma_start(out=tt_out[:, 0:CB], in_=tout_l[:, :])
                    nc.gpsimd.dma_start(out=zcat_out[:, 0:CB],
                                        in_=zc_sb_l[:, :])
                elif p1 == BATCH:
                    nc.vector.tensor_tensor(
                        tout_r[:, :], tpb[:, :], tacc_r[:, :], ADD)
                    nc.scalar.copy(zc_sb_r[:, :], zc_ps[1][:, :])
                    nc.sync.dma_start(out=tt_out[:, CB:BATCH], in_=tout_r[:, :])
                    nc.gpsimd.dma_start(out=zcat_out[:, CB:BATCH],
                                        in_=zc_sb_r[:, :])
    return nc


def _prepare(inputs):
    features = np.asarray(inputs["features"], dtype=np.float32)
    labels = np.asarray(inputs["labels"]).astype(np.int64)
    groups = np.asarray(inputs["groups"]).astype(np.int64)
    weights = np.asarray(inputs["weights"], dtype=np.float32)
    noise = np.asarray(inputs["noise"], dtype=np.float32)
    group_embed = np.asarray(inputs["group_embed"], dtype=np.float32)
    W1 = np.asarray(inputs["W1"], dtype=np.float32)
    b1 = np.asarray(inputs["b1"], dtype=np.float32)
    W2 = np.asarray(inputs["W2"], dtype=np.float32)
    b2 = np.asarray(inputs["b2"], dtype=np.float32)
    freq_factors = np.asarray(inputs["freq_factors"], dtype=np.float32)
    roi_1_factors = np.asarray(inputs["roi_1_factors"], dtype=np.float32)
    roi_2_factors = np.asarray(inputs["roi_2_factors"], dtype=np.float32)
    lin_W = np.asarray(inputs["lin_W"], dtype=np.float32)
    lin_b = np.asarray(inputs["lin_b"], dtype=np.float32)
    logit_bias = np.asarray(inputs["logit_bias"], dtype=np.float32)

    b = features.shape[0]
    flat = features.reshape(b, -1)

    perm = np.argsort(groups, kind="stable")
    groups_sorted = groups[perm]
    blocks = _quad_blocks(groups_sorted)

    sq = np.einsum("bi,bi->b", flat, flat, optimize=True)

    flat_q = flat[perm].astype(NPFP8)
    flatT = flat_q.view(np.uint8).T.copy().view(NPFP8)      # [N_FEAT, BATCH]

    W = (np.concatenate([W1[:N_FEAT], W2[:N_FEAT]], axis=1) * W_SCALE).astype(NPFP8)

    Ff = _softplus(freq_factors)
    R1 = _softplus(roi_1_factors)
    R2 = _softplus(roi_2_factors)

    # R12[(r,s), (g,z)] in fp8 with a dynamic power-of-two scale.
    R12 = (R1[:, :, :, None] * R2[:, :, None, :])           # [g, z, r, s]
    # ml_dtypes.float8_e4m3 is the IEEE variant: max finite value is 240
    r_scale = float(2.0 ** np.floor(np.log2(224.0 / max(float(R12.max()), 1e-6))))
    R12q = (R12 * r_scale).transpose(2, 3, 0, 1).reshape(RS, N_GROUPS * Z)
    R12q = R12q.astype(NPFP8)                               # [rs, gz]
    r12_dev = R12q.view(np.uint8).reshape(4, 2, 128, N_GROUPS * Z)
    r12_dev = r12_dev.transpose(2, 0, 1, 3).copy().view(NPFP8)

    # ff[i, f, Q] = Ff[g, z, f]/Ff[g, z, f_last] with g = 4Q+i//32, z = i%32
    # (renormalized so the device's last fold is a plain add; the host folds
    # the Ff[g, z, f_last]/r_scale factor back in when decoding tt). f_last
    # is per-core: the core's highest frequency index.
    ff_all = np.zeros((N_CORES, 128, F_PER_CORE, N_QUADS), np.float32)
    ff_last = np.zeros((N_CORES, N_GROUPS, Z), np.float32)
    for c in range(N_CORES):
        fglob = np.arange(F_PER_CORE) + c * F_PER_CORE
        ff_last[c] = Ff[:, :, fglob[-1]]
        for q in range(N_GROUPS // 4):
            for gp in range(4):
                g = 4 * q + gp
                p0 = gp * 32
                # Ff[g, :, fglob] has shape (f, z) via advanced indexing
                ff_all[c, p0:p0 + 32, :, q] = (
                    Ff[g, :, fglob] / ff_last[c, g][None, :]).T

    in_maps = []
    for c in range(N_CORES):
        fsl = flatT[c * FEAT_PER_CORE:(c + 1) * FEAT_PER_CORE]
        fsl = fsl.view(np.uint8).reshape(KPAIRS, 2, 128, BATCH)
        fsl = fsl.transpose(2, 0, 1, 3).copy().view(NPFP8)
        # piece-major layout of the tail kps for the contiguous tail DMAs
        tail = fsl[:, TAIL_KP0:TAIL_KP0 + TAIL_NKP, :, :].view(np.uint8)
        tail = np.concatenate(
            [tail[:, :, :, p0:p1].reshape(128, -1) for (p0, p1) in TAIL_PIECES],
            axis=1).copy().view(NPFP8)
        wsl = W[c * FEAT_PER_CORE:(c + 1) * FEAT_PER_CORE].view(np.uint8)
        wsl = wsl.reshape(KPAIRS, 2, 128, 64).transpose(2, 0, 1, 3).copy().view(NPFP8)
        in_maps.append({
            "flatt": fsl,
            "flat_tail": tail,
            "w": wsl,
            "r12": r12_dev,
            "ff": ff_all[c],
        })

    host = dict(
        labels=labels, groups=groups, weights=weights, noise=noise,
        group_embed=group_embed, W1=W1, b1=b1, W2=W2, b2=b2,
        lin_W=lin_W, lin_b=lin_b, logit_bias=logit_bias,
        Ff=Ff, R1=R1, R2=R2, sq=sq, perm=perm, b=b,
        groups_sorted=groups_sorted, ff_last=ff_last, r_scale=r_scale,
    )
    return in_maps, blocks, host


def _finish(zcatT, T_sorted, host):
    b = host["b"]
    perm = host["perm"]
    inv = np.empty_like(perm)
    inv[perm] = np.arange(b)

    zcat = (zcatT / W_SCALE).T[inv]                        # [b, 64]
    T = T_sorted[inv]

    groups = host["groups"]
    ge = host["group_embed"][groups]
    z_mu = zcat[:, :Z] + host["b1"] + ge @ host["W1"][N_FEAT:]
    z_log_std = zcat[:, Z:] + host["b2"] + ge @ host["W2"][N_FEAT:]
    sigma = EPSILON + np.exp(z_log_std)
    kld = np.sum(-np.log(sigma) + 0.5 * (sigma * sigma + z_mu * z_mu - 1.0), axis=1)
    zs = z_mu + sigma * host["noise"]
    zs = zs @ host["lin_W"] + host["lin_b"]
    a = _softplus(zs)

    Ff, R1, R2 = host["Ff"], host["R1"], host["R2"]
    M = (np.einsum("gzf,gyf->gzy", Ff, Ff)
         * np.einsum("gzr,gyr->gzy", R1, R1)
         * np.einsum("gzs,gys->gzy", R2, R2))
    vol2 = np.einsum("bz,bzy,by->b", a, M[groups], a)
    fdotv = np.sum(a * T, axis=1)
    rec = REG_STRENGTH * (host["sq"] - 2.0 * fdotv + vol2) / N_FEAT

    logits = np.concatenate([zs[:, :N_CLASSES - 1], np.ones((b, 1), np.float32)],
                            axis=1) + host["logit_bias"]
    m = logits.max(axis=1, keepdims=True)
    lse = m[:, 0] + np.log(np.exp(logits - m).sum(axis=1))
    log_probs = logits[np.arange(b), host["labels"]] - lse

    freq_loss = np.var(Ff, axis=0, ddof=1).mean(axis=1).sum()
    roi_loss = (np.var(R1, axis=0, ddof=1) + np.var(R2, axis=0, ddof=1)).mean(axis=1).sum()

    loss = np.mean(rec - host["weights"] * log_probs + KL_FACTOR * kld) \
        + freq_loss + roi_loss
    return np.float32(loss)


def kernel(**inputs) -> np.ndarray:
    _install_waitfix()
    in_maps, blocks, host = _prepare(inputs)
    nc = build_device_program(blocks)
    r = run_bass_kernel_spmd(nc, in_maps, core_ids=list(range(N_CORES)))
    gs = host["groups_sorted"]
    gs4 = (gs % 4).astype(np.int64)
    cols = np.arange(host["b"])
    zcatT = np.zeros((64, BATCH), np.float32)
    T_sorted = np.zeros((host["b"], Z), np.float32)
    for c in range(N_CORES):
        zcatT += r.results[c]["zcat"].astype(np.float32)
        ttc = r.results[c]["tt"].astype(np.float32)
        # undo the per-core Ff renormalization: x Ff[g, z, f_last]/r_scale
        scale = host["ff_last"][c][gs]                     # [b, Z]
        for zi in range(Z):
            T_sorted[:, zi] += ttc[gs4 * Z + zi, cols] * scale[:, zi]
    T_sorted /= host["r_scale"]
    return _finish(zcatT, T_sorted, host)
